# revision 2
# baseline (speedup 1.0000x reference)
"""Trainium2 Bass kernel for nn_BinaryDiff: out = x @ base + coeff * (x @ mask).

Fused as a single matmul: out = x @ W where W = base + coeff * mask.

Sharding over 8 NeuronCores: 4 row-groups of x (2048 rows each) x 2
column-groups of W (2048 cols each). Each core computes a [2048, 2048]
tile of the [8192, 4096] output.

Per-core device kernel:
  - W = base + c*mask built on-chip (DVE), cached in SBUF as bf16
    [128, 32, 2048] (k-major tiles).
  - x row-blocks staged in halves [128, 2048] f32, transposed 128x128 at a
    time on the TensorE (fp32 transpose-mode), cast to bf16 on PSUM->SBUF
    copyback (DVE).
  - matmul: for each m-tile, 32 k-tiles x 4 n-tiles of [128,128]@[128,512]
    bf16 matmuls accumulating fp32 in PSUM.
  - PSUM->SBUF output copyback on ScalarE, DMA out as fp32.
"""

import numpy as np

import concourse.bass as bass
import concourse.mybir as mybir
import concourse.tile as tile
from concourse import bacc
from concourse.masks import make_identity

P = 128
FULL_M, FULL_K, FULL_N = 8192, 4096, 4096
ROW_SHARDS, COL_SHARDS = 4, 2
CORE_M = FULL_M // ROW_SHARDS   # 2048
CORE_N = FULL_N // COL_SHARDS   # 2048


def build_kernel(M=CORE_M, K=FULL_K, N=CORE_N, debug=False):
    """Build the per-core Bass program. All cores run the same program (SPMD)."""
    f32 = mybir.dt.float32
    i32 = mybir.dt.int32
    bf16 = mybir.dt.bfloat16

    M_T = M // P            # m-tiles of 128 rows
    K_T = K // P            # k-tiles of 128
    N_MM = 512              # matmul moving free dim (one PSUM bank)
    N_T = N // N_MM
    WCH = min(N, 1024)      # W-build chunk width
    WCH_T = N // WCH
    XH = min(K, 2048)       # x staging half-width
    XH_T = K // XH

    nc = bacc.Bacc("TRN2", target_bir_lowering=False, debug=debug)

    x_d = nc.dram_tensor("x", [M, K], f32, kind="ExternalInput").ap()
    base_d = nc.dram_tensor("base", [K, N], f32, kind="ExternalInput").ap()
    mask_d = nc.dram_tensor("mask", [K, N], i32, kind="ExternalInput").ap()
    coeff_d = nc.dram_tensor("coeff", [P, 1], f32, kind="ExternalInput").ap()
    out_d = nc.dram_tensor("out", [M, N], f32, kind="ExternalOutput").ap()

    with tile.TileContext(nc) as tc:
        with (
            tc.tile_pool(name="const", bufs=1) as const,
            tc.tile_pool(name="wcache", bufs=1) as wcache,
            tc.tile_pool(name="wstage", bufs=2) as wstage,
            tc.tile_pool(name="xstage", bufs=2) as xstage,
            tc.tile_pool(name="xt", bufs=2) as xtpool,
            tc.tile_pool(name="ostage", bufs=3) as ostage,
            tc.tile_pool(name="tpsum", bufs=2, space="PSUM") as tpsum,
            tc.tile_pool(name="mpsum", bufs=4, space="PSUM") as mpsum,
        ):
            ident = const.tile([P, P], f32)
            make_identity(nc, ident[:])
            c128 = const.tile([P, 1], f32)
            nc.sync.dma_start(out=c128[:], in_=coeff_d[:])

            # ---- Build W = base + c*mask in SBUF (bf16), k-tile-major ----
            w_sb = wcache.tile([P, K_T, N], bf16)
            for k in range(K_T):
                for h in range(WCH_T):
                    cs = slice(h * WCH, (h + 1) * WCH)
                    bst = wstage.tile([P, WCH], f32, name="bst")
                    mst = wstage.tile([P, WCH], i32, name="mst")
                    nc.sync.dma_start(out=bst[:], in_=base_d[k * P:(k + 1) * P, cs])
                    nc.sync.dma_start(out=mst[:], in_=mask_d[k * P:(k + 1) * P, cs])
                    mf = wstage.tile([P, WCH], f32, name="mf")
                    nc.vector.tensor_copy(out=mf[:], in_=mst[:])
                    # W = (mf * c) + base
                    nc.vector.scalar_tensor_tensor(
                        out=w_sb[:, k, cs],
                        in0=mf[:],
                        scalar=c128[:, 0:1],
                        in1=bst[:],
                        op0=mybir.AluOpType.mult,
                        op1=mybir.AluOpType.add,
                    )

            # ---- Main loop over m-tiles ----
            for m in range(M_T):
                rs = slice(m * P, (m + 1) * P)
                # stage x rows, transpose to [K, 128] bf16
                xt = xtpool.tile([P, K_T, P], bf16)
                for h in range(XH_T):
                    xst = xstage.tile([P, XH], f32, name="xst")
                    nc.sync.dma_start(
                        out=xst[:], in_=x_d[rs, h * XH:(h + 1) * XH]
                    )
                    for kk in range(XH // P):
                        k = h * (XH // P) + kk
                        pst = tpsum.tile([P, P], f32)
                        nc.tensor.transpose(
                            pst[:], xst[:, kk * P:(kk + 1) * P], ident[:]
                        )
                        nc.vector.tensor_copy(out=xt[:, k, :], in_=pst[:])

                # matmuls: accumulate over k into N_T psum banks
                psums = [
                    mpsum.tile([P, N_MM], f32, name="mmps") for _ in range(N_T)
                ]
                for k in range(K_T):
                    for n in range(N_T):
                        nc.tensor.matmul(
                            psums[n][:],
                            lhsT=xt[:, k, :],
                            rhs=w_sb[:, k, n * N_MM:(n + 1) * N_MM],
                            start=(k == 0),
                            stop=(k == K_T - 1),
                        )
                for n in range(N_T):
                    ob = ostage.tile([P, N_MM], f32, name="ob")
                    nc.scalar.copy(out=ob[:], in_=psums[n][:])
                    nc.sync.dma_start(
                        out=out_d[rs, n * N_MM:(n + 1) * N_MM], in_=ob[:]
                    )

    nc.compile()
    return nc


_NC_CACHE = {}


def _get_nc():
    if "nc" not in _NC_CACHE:
        _NC_CACHE["nc"] = build_kernel()
    return _NC_CACHE["nc"]


def make_in_maps(x, base, coeff, mask):
    x = np.asarray(x, dtype=np.float32)
    base = np.asarray(base, dtype=np.float32)
    mask = np.asarray(mask, dtype=np.int32)
    coeff = np.asarray(coeff, dtype=np.float32)

    B, L, D_IN = x.shape
    x2 = np.ascontiguousarray(x.reshape(B * L, D_IN))
    c128 = np.full((P, 1), coeff[0], dtype=np.float32)

    in_maps = []
    for i in range(8):
        rg, cg = i // COL_SHARDS, i % COL_SHARDS
        in_maps.append(
            {
                "x": x2[rg * CORE_M:(rg + 1) * CORE_M],
                "base": np.ascontiguousarray(
                    base[:, cg * CORE_N:(cg + 1) * CORE_N]
                ),
                "mask": np.ascontiguousarray(
                    mask[:, cg * CORE_N:(cg + 1) * CORE_N]
                ),
                "coeff": c128,
            }
        )
    return in_maps, (B, L)


def assemble(results, B, L):
    out = np.empty((B * L, FULL_N), dtype=np.float32)
    for i in range(8):
        rg, cg = i // COL_SHARDS, i % COL_SHARDS
        out[rg * CORE_M:(rg + 1) * CORE_M, cg * CORE_N:(cg + 1) * CORE_N] = (
            results[i]["out"]
        )
    return out.reshape(B, L, FULL_N)


def kernel(x, base, coeff, mask):
    from concourse.bass_utils import run_bass_kernel_spmd

    in_maps, (B, L) = make_in_maps(x, base, coeff, mask)
    nc = _get_nc()
    res = run_bass_kernel_spmd(nc, in_maps, list(range(8)))
    return assemble(res.results, B, L)


# revision 3
# speedup vs baseline: 1.1495x; 1.1495x over previous
"""Trainium2 Bass kernel for nn_BinaryDiff: out = x @ base + coeff * (x @ mask).

Fused as a single matmul: out = x @ W where W = base + coeff * mask.

Sharding over 8 NeuronCores: 4 row-groups of x (2048 rows each) x 2
column-groups of W (2048 cols each). Each core computes a [2048, 2048]
tile of the [8192, 4096] output.

Per-core device kernel (two N-half passes to hide the 64 MiB W load):
  - W = base + c*mask built on-chip (one DVE scalar_tensor_tensor per
    k-tile, int32 mask consumed directly), cached in SBUF as bf16.
    Half A (cols 0:N/2) is built up front; half B's loads+builds are
    emitted interleaved into PASS A so they stream during compute.
  - PASS A: per m-tile, stage x rows fp32, transpose 128x128 blocks on
    TensorE, cast to bf16 on the PSUM->SBUF copyback (DVE), spill the
    transposed tile to DRAM, then matmul against W-half-A.
  - PASS B: stream the spilled xT tiles back (1 DMA each) and matmul
    against W-half-B. No transposes.
  - PSUM fp32 accumulation over K; output copyback on ScalarE; fp32 out.
"""

import numpy as np

import concourse.bass as bass
import concourse.mybir as mybir
import concourse.tile as tile
from concourse import bacc
from concourse.masks import make_identity

P = 128
FULL_M, FULL_K, FULL_N = 8192, 4096, 4096
ROW_SHARDS, COL_SHARDS = 4, 2
CORE_M = FULL_M // ROW_SHARDS   # 2048
CORE_N = FULL_N // COL_SHARDS   # 2048


def build_kernel(M=CORE_M, K=FULL_K, N=CORE_N, debug=False):
    """Build the per-core Bass program. All cores run the same program (SPMD)."""
    f32 = mybir.dt.float32
    i32 = mybir.dt.int32
    bf16 = mybir.dt.bfloat16

    M_T = M // P            # m-tiles of 128 rows
    K_T = K // P            # k-tiles of 128
    N_MM = 512              # matmul moving free dim (one PSUM bank)
    NH = N // 2             # N half width
    NH_T = NH // N_MM       # 512-subtiles per half
    XH = min(K, 2048)       # x staging half-width
    XH_T = K // XH

    nc = bacc.Bacc("TRN2", target_bir_lowering=False, debug=debug)

    x_d = nc.dram_tensor("x", [M, K], f32, kind="ExternalInput").ap()
    base_d = nc.dram_tensor("base", [K, N], f32, kind="ExternalInput").ap()
    mask_d = nc.dram_tensor("mask", [K, N], i32, kind="ExternalInput").ap()
    coeff_d = nc.dram_tensor("coeff", [P, 1], f32, kind="ExternalInput").ap()
    out_d = nc.dram_tensor("out", [M, N], f32, kind="ExternalOutput").ap()

    with tile.TileContext(nc) as tc:
        with (
            tc.tile_pool(name="const", bufs=1) as const,
            tc.tile_pool(name="wcache", bufs=1) as wcache,
            tc.tile_pool(name="wstage", bufs=2) as wstage,
            tc.tile_pool(name="xstage", bufs=2) as xstage,
            tc.tile_pool(name="xt", bufs=3) as xtpool,
            tc.tile_pool(name="ostage", bufs=3) as ostage,
            tc.tile_pool(name="xspill", bufs=1, space="DRAM") as xspill,
            tc.tile_pool(name="tpsum", bufs=2, space="PSUM") as tpsum,
            tc.tile_pool(name="mpsum", bufs=6, space="PSUM") as mpsum,
        ):
            ident = const.tile([P, P], f32)
            make_identity(nc, ident[:])
            c128 = const.tile([P, 1], f32)
            nc.sync.dma_start(out=c128[:], in_=coeff_d[:])

            w_a = wcache.tile([P, K_T, NH], bf16, name="w_a")
            w_b = wcache.tile([P, K_T, NH], bf16, name="w_b")
            xts = xspill.tile([M_T, P, K_T * P], bf16)

            def build_w_chunk(k, half):
                """Load base/mask k-tile for one N-half and fuse into W."""
                cs = slice(half * NH, (half + 1) * NH)
                dst = w_a if half == 0 else w_b
                bst = wstage.tile([P, NH], f32, name="bst")
                mst = wstage.tile([P, NH], i32, name="mst")
                nc.sync.dma_start(out=bst[:], in_=base_d[k * P:(k + 1) * P, cs])
                nc.sync.dma_start(out=mst[:], in_=mask_d[k * P:(k + 1) * P, cs])
                nc.vector.scalar_tensor_tensor(
                    out=dst[:, k, :],
                    in0=mst[:],
                    scalar=c128[:, 0:1],
                    in1=bst[:],
                    op0=mybir.AluOpType.mult,
                    op1=mybir.AluOpType.add,
                )

            def mm_group(xt, w_half, m, half):
                """All matmuls + copyback + out DMA for one (m-tile, N-half)."""
                rs = slice(m * P, (m + 1) * P)
                psums = [
                    mpsum.tile([P, N_MM], f32, name="mmps") for _ in range(NH_T)
                ]
                for k in range(K_T):
                    for n in range(NH_T):
                        nc.tensor.matmul(
                            psums[n][:],
                            lhsT=xt[:, k, :],
                            rhs=w_half[:, k, n * N_MM:(n + 1) * N_MM],
                            start=(k == 0),
                            stop=(k == K_T - 1),
                        )
                for n in range(NH_T):
                    ob = ostage.tile([P, N_MM], f32, name="ob")
                    nc.scalar.copy(out=ob[:], in_=psums[n][:])
                    col0 = half * NH + n * N_MM
                    nc.scalar.dma_start(
                        out=out_d[rs, col0:col0 + N_MM], in_=ob[:]
                    )

            # ---- W half A up front ----
            for k in range(K_T):
                build_w_chunk(k, 0)

            # ---- PASS A: transpose x, spill xT, matmul vs W-half-A ----
            # W half B chunks are emitted inside the loop so they stream
            # during PASS A compute.
            wb_per_m = (K_T + M_T - 1) // M_T
            for m in range(M_T):
                rs = slice(m * P, (m + 1) * P)
                xt = xtpool.tile([P, K_T, P], bf16)
                for h in range(XH_T):
                    xst = xstage.tile([P, XH], f32, name="xst")
                    nc.gpsimd.dma_start(
                        out=xst[:], in_=x_d[rs, h * XH:(h + 1) * XH]
                    )
                    for kk in range(XH // P):
                        k = h * (XH // P) + kk
                        pst = tpsum.tile([P, P], f32)
                        nc.tensor.transpose(
                            pst[:], xst[:, kk * P:(kk + 1) * P], ident[:]
                        )
                        nc.vector.tensor_copy(out=xt[:, k, :], in_=pst[:])
                nc.gpsimd.dma_start(out=xts[m], in_=xt[:])
                mm_group(xt, w_a, m, 0)
                for j in range(wb_per_m):
                    k = m * wb_per_m + j
                    if k < K_T:
                        build_w_chunk(k, 1)

            # ---- PASS B: stream xT back, matmul vs W-half-B ----
            for m in range(M_T):
                xt = xtpool.tile([P, K_T, P], bf16)
                nc.gpsimd.dma_start(out=xt[:], in_=xts[m])
                mm_group(xt, w_b, m, 1)

    nc.compile()
    return nc


_NC_CACHE = {}


def _get_nc():
    if "nc" not in _NC_CACHE:
        _NC_CACHE["nc"] = build_kernel()
    return _NC_CACHE["nc"]


def make_in_maps(x, base, coeff, mask):
    x = np.asarray(x, dtype=np.float32)
    base = np.asarray(base, dtype=np.float32)
    mask = np.asarray(mask, dtype=np.int32)
    coeff = np.asarray(coeff, dtype=np.float32)

    B, L, D_IN = x.shape
    x2 = np.ascontiguousarray(x.reshape(B * L, D_IN))
    c128 = np.full((P, 1), coeff[0], dtype=np.float32)

    in_maps = []
    for i in range(8):
        rg, cg = i // COL_SHARDS, i % COL_SHARDS
        in_maps.append(
            {
                "x": x2[rg * CORE_M:(rg + 1) * CORE_M],
                "base": np.ascontiguousarray(
                    base[:, cg * CORE_N:(cg + 1) * CORE_N]
                ),
                "mask": np.ascontiguousarray(
                    mask[:, cg * CORE_N:(cg + 1) * CORE_N]
                ),
                "coeff": c128,
            }
        )
    return in_maps, (B, L)


def assemble(results, B, L):
    out = np.empty((B * L, FULL_N), dtype=np.float32)
    for i in range(8):
        rg, cg = i // COL_SHARDS, i % COL_SHARDS
        out[rg * CORE_M:(rg + 1) * CORE_M, cg * CORE_N:(cg + 1) * CORE_N] = (
            results[i]["out"]
        )
    return out.reshape(B, L, FULL_N)


def kernel(x, base, coeff, mask):
    from concourse.bass_utils import run_bass_kernel_spmd

    in_maps, (B, L) = make_in_maps(x, base, coeff, mask)
    nc = _get_nc()
    res = run_bass_kernel_spmd(nc, in_maps, list(range(8)))
    return assemble(res.results, B, L)
